# revision 1
# baseline (speedup 1.0000x reference)
"""Tensor-parallel GQA attention (sigmoid-gated) for Trainium2, 8 NeuronCores.

Problem: B=2, S=2048, D=2048, H=32 q-heads, KV=8 kv-heads, HD=64 (GQA groups=4),
RoPE on q/k, full (non-causal) softmax, sigmoid(gate) output gating, out proj.

Sharding (tensor-parallel over heads): core c owns q-heads 4c..4c+3, kv-head c,
the matching 256 q-cols + 256 gate-cols of Wq, 64-col slices of Wk/Wv, and rows
256c:256c+256 of Wo. Each core computes a full [B*S, D] partial of the output
projection; the host sums the 8 partials.

Per-core pipeline (projection/attn-prob/out-proj matmuls in bf16, scores in
float32r; rel-L2 error ~1e-2 vs the 2e-2 gate):
  A) projections psum[m,t] += W[d,m].T @ hsT[d,t], one 2MB hsT DMA per
     512-column chunk (DMA-issue cadence is a bottleneck, not bandwidth).
     q lands in head-PAIR layout qP[128, 2, T] (head 2m at partitions 0:64,
     head 2m+1 at 64:128); k is evicted to kv2lo (rows 0:64, zeros above) and
     mirrored into kv2hi (rows 64:128, zeros below) so each head's scores
     contract K=128 against the zero-padded copy matching its partition half.
     RoPE is fused per chunk: rotate-half runs as a PE permutation matmul
     into PSUM (walrus forbids partition-shifted SBUF+SBUF TensorTensor),
     then three partition-aligned DVE ops finish x*cos + rot*sin.  Gate
     columns get sigmoid at eviction into a resident bf16 SBUF tile.
  B) attention in scoresT orientation over 1024-wide i-halves:
     scoresT[j,i] = kv[:,j].T @ qP[:,i] (two bank-aligned 512 psum writes),
     probs = exp(s/8) via ACT activation scale=0.125 into bf16 for most
     j-chunks and via a 1-op DVE Schraudolph bit-trick (int32 tensor_scalar;
     the top 16 bits ARE the bf16 value, consumed through a stride-2 bf16
     view) for DVE_JCS chunks -- splits the activation load across engines;
     softmax renormalization absorbs most of the approximation error.
     attnT[hd,i] += v1[j, hd|1].T @ probs in bf16; v1's ones-column
     accumulates denominators in psum row 64 for free.  Normalization is
     deferred one i-half (runs under the next one's matmuls): denom row ->
     SBUF, broadcast via a row-64-selector matmul, reciprocal_approx_fast
     (rows 0:64 only -- the custom op misbehaves at base partition 64),
     then a gated mul on DVE and the sigmoid-gate mul on the Pool engine.
  C) out[t,dout] += attnG[m,t].T @ Wo[m,dout] bf16 partials written as bf16,
     sharing the psat psum ring, interleaved per 4-t-tile chunks between
     batch-1 heads so eviction load spreads.
"""

import sys

sys.path.insert(0, "/opt/trn_rl_repo")

import numpy as np

import concourse.bass as bass  # noqa: F401
import concourse.mybir as mybir
import concourse.tile as tile
from concourse import bacc
from concourse.bass_utils import run_bass_kernel_spmd

F32 = mybir.dt.float32
F32R = mybir.dt.float32r
I32 = mybir.dt.int32
BF16 = mybir.dt.bfloat16
AF = mybir.ActivationFunctionType
ALU = mybir.AluOpType

P = 128
B, S, D = 2, 2048, 2048
T = B * S                  # 4096 token rows (batch folded)
H, KV, HD = 32, 8, 64
HH = HD // 2               # 32
NCORES = 8
NH = H // NCORES           # 4 q-heads per core
MQ = NH * HD               # 256 q-cols per core
DC = D // P                # 16 contraction chunks
TCH = 512                  # moving-dim chunk
NTCH = T // TCH            # 8
SJ = S // P                # 16 key chunks per batch
NSEG = S // TCH            # 4 i-segments per batch
NT = T // P                # 32 t-tiles
NP = SJ // 2               # 8 j-pairs per segment

# Schraudolph fast-exp (DVE): exp(s/8) ~= bitcast_i32_f32(s*AS + BS)
AS_CONST = float((1 << 23) * 1.4426950408889634 * 0.125)
BS_CONST = float(127 * (1 << 23) - 449200)
# which j-chunks (of 16 per i-half) compute probs on DVE instead of ACT
DVE_JCS = (3, 7, 11)


def build_nc(nreps=1):
    nc = bacc.Bacc("TRN2", target_bir_lowering=False, debug=False)

    hsT = nc.dram_tensor("hsT", [D, T], BF16, kind="ExternalInput")
    wqg = nc.dram_tensor("wqg", [D, 2 * MQ], BF16, kind="ExternalInput")
    wkv = nc.dram_tensor("wkv", [D, 2 * HD], BF16, kind="ExternalInput")
    wo = nc.dram_tensor("wo", [MQ, D], BF16, kind="ExternalInput")
    # rope tables, [128, S] with rows duplicated (row p holds entry p % 64)
    ck = nc.dram_tensor("ck", [P, S], F32, kind="ExternalInput")   # cos
    sk = nc.dram_tensor("sk", [P, S], F32, kind="ExternalInput")   # signed sin
    identd = nc.dram_tensor("ident", [HD, HD], F32R, kind="ExternalInput")
    oseld = nc.dram_tensor("osel", [P, P], F32R, kind="ExternalInput")  # row64=1
    protd = nc.dram_tensor("prot", [P, P], F32R, kind="ExternalInput")  # xor-32 perm
    onesd = nc.dram_tensor("ones", [P, B * SJ], BF16, kind="ExternalInput")
    out = nc.dram_tensor("out", [T, D], BF16, kind="ExternalOutput")

    hsT3 = hsT.ap().rearrange("(o p) t -> p o t", p=P)   # [128, 16, 4096]
    wqg3 = wqg.ap().rearrange("(o p) m -> p o m", p=P)   # [128, 16, 512]
    wkv3 = wkv.ap().rearrange("(o p) m -> p o m", p=P)   # [128, 16, 128]
    wo3 = wo.ap().rearrange("(o p) n -> p o n", p=P)     # [128, 2, 2048]

    with tile.TileContext(nc) as tc:
        for _rep in range(nreps):
            with (
                tc.tile_pool(name="const", bufs=1) as const,
                tc.tile_pool(name="big", bufs=1) as big,
            ):
                ident_sb = const.tile([HD, HD], F32R)
                osel_sb = const.tile([P, P], F32R)  # row 64 ones, rest zero
                prot_sb = const.tile([P, P], F32R)  # xor-32 permutation
                nc.sync.dma_start(ident_sb[:], identd.ap())
                nc.sync.dma_start(osel_sb[:], oseld.ap())
                nc.sync.dma_start(prot_sb[:], protd.ap())

                # ---- persistent activations ----
                qP_sb = big.tile([P, 2, T], F32R)    # head pairs, roped in place
                kv2lo = big.tile([P, T], F32R)       # roped k rows 0:64, 0 above
                kv2hi = big.tile([P, T], F32R)       # 0 below, roped k rows 64:128
                v1_sb = big.tile([P, B * SJ, P], BF16)  # v | ones col | zeros
                gsb = big.tile([P, 2, T], BF16)      # sigmoid(gate), resident

                # ---- stage A: projections + fused rope ----
                with (
                    nc.named_scope("stageA"),
                    tc.tile_pool(name="wpool", bufs=1) as wpool,
                    tc.tile_pool(name="tab", bufs=1) as tab,
                    tc.tile_pool(name="hst", bufs=3) as hst_pool,
                    tc.tile_pool(name="vst", bufs=8) as vst,
                    tc.tile_pool(name="ps512", bufs=5, space="PSUM") as ps512,
                    tc.tile_pool(name="psrot", bufs=3, space="PSUM") as psrot,
                ):
                    ck_sb = tab.tile([P, S], F32)
                    sk_sb = tab.tile([P, S], F32)

                    wqg_sb = wpool.tile([P, DC, 2 * MQ], BF16)
                    wkv_sb = wpool.tile([P, DC, 2 * HD], BF16)
                    # zero pads: scores contract K=128 with a zeroed half so
                    # each head pair partition-half stays independent
                    nc.gpsimd.memset(kv2lo[HD:P, :].bitcast(F32), 0.0)
                    nc.gpsimd.memset(kv2hi[0:HD, :].bitcast(F32), 0.0)
                    # v1 cols 65:128 zero -> psum rows 65:128 stay finite zeros
                    nc.gpsimd.memset(v1_sb[:, :, HD + 1:P], 0.0)

                    def rope(x, tabsl, rows):
                        # x: [128, TCH] slice, roped in place on rows [0:rows].
                        # rot_half via PE permutation matmul (psum), then only
                        # partition-aligned TTs: rot*=sin; x*=cos; x+=rot.
                        rot = psrot.tile([P, TCH], F32, tag="rot")
                        nc.tensor.matmul(
                            rot[:], lhsT=prot_sb[:], rhs=x, start=True, stop=True)
                        c = ck_sb[:, tabsl]
                        s = sk_sb[:, tabsl]
                        xr = x[0:rows, :] if rows < P else x
                        nc.vector.tensor_mul(
                            out=rot[0:rows, :], in0=rot[0:rows, :],
                            in1=s[0:rows, :])
                        nc.vector.tensor_mul(out=xr, in0=xr, in1=c[0:rows, :])
                        nc.vector.tensor_add(out=xr, in0=xr, in1=rot[0:rows, :])

                    vstgs = []
                    pend_rope = []
                    for tci in range(NTCH):
                        ts = slice(tci * TCH, (tci + 1) * TCH)
                        t0 = (tci * TCH) % S
                        tabsl = slice(t0, t0 + TCH)
                        if tci == 0:
                            nc.sync.dma_start(wkv_sb[:], wkv3)
                            for dq in range(4):
                                nc.sync.dma_start(
                                    wqg_sb[:, dq * 4:(dq + 1) * 4, :],
                                    wqg3[:, dq * 4:(dq + 1) * 4, :])
                        ht = hst_pool.tile([P, DC * TCH], BF16, tag="hst")
                        hview = ht[:].rearrange("p (o t) -> p o t", o=DC)
                        if tci == 0:
                            for db in range(4):
                                nc.sync.dma_start(
                                    hview[:, db * 4:(db + 1) * 4, :],
                                    hsT3[:, db * 4:(db + 1) * 4, ts])
                        else:
                            nc.sync.dma_start(hview, hsT3[:, :, ts])
                        if tci == 0:
                            nc.sync.dma_start(ck_sb[:], ck.ap())
                            nc.sync.dma_start(sk_sb[:], sk.ap())
                        pss = [ps512.tile([P, TCH], F32, tag="ps512",
                                          name=f"psA{_m}") for _m in range(5)]
                        for dc in range(DC):
                            for mt in range(5):  # 0: kv, 1-2: q pairs, 3-4: gate
                                if mt == 0:
                                    w = wkv_sb[:, dc, :]
                                else:
                                    w = wqg_sb[:, dc, (mt - 1) * P:mt * P]
                                nc.tensor.matmul(
                                    pss[mt][:],
                                    lhsT=w,
                                    rhs=ht[:, dc * TCH:(dc + 1) * TCH],
                                    start=(dc == 0),
                                    stop=(dc == DC - 1),
                                )
                        if pend_rope:
                            pend_rope.pop(0)()
                        # evictions
                        nc.vector.tensor_copy(kv2lo[0:HD, ts], pss[0][0:HD, :])
                        vstg = vst.tile([HD, TCH], F32R, tag="vst")
                        nc.vector.tensor_copy(vstg[:], pss[0][HD:P, :])
                        nc.scalar.copy(qP_sb[:, 0, ts], pss[1][:])
                        nc.scalar.copy(qP_sb[:, 1, ts], pss[2][:])
                        for mo in range(2):
                            nc.scalar.activation(gsb[:, mo, ts], pss[3 + mo][:],
                                                 AF.Sigmoid)
                        # rope deferred one chunk: the rot matmuls land
                        # in the PE queue after the NEXT chunk's projection
                        # matmuls, whose evictions are then long done
                        def do_rope(ts=ts, tabsl=tabsl):
                            rope(kv2lo[:, ts], tabsl, HD)
                            nc.vector.tensor_copy(kv2hi[HD:P, ts],
                                                  kv2lo[0:HD, ts])
                            rope(qP_sb[:, 0, ts], tabsl, P)
                            rope(qP_sb[:, 1, ts], tabsl, P)
                        pend_rope.append(do_rope)
                        vstgs.append(vstg)
                    while pend_rope:
                        pend_rope.pop(0)()
                    # batched v transposes: PE transpose-mode entered once
                    for tci in range(NTCH):
                        for j4 in range(TCH // P):
                            jc = tci * (TCH // P) + j4
                            vt_ps = psrot.tile([P, HD], F32R, tag="rot")
                            nc.tensor.transpose(
                                vt_ps[:],
                                vstgs[tci][:, j4 * P:(j4 + 1) * P],
                                ident_sb[:],
                            )
                            nc.scalar.copy(v1_sb[:, jc, 0:HD], vt_ps[:])
                    nc.sync.dma_start(v1_sb[:, :, HD:HD + 1], onesd.ap()[:, :, None])

                # ---- stage B: attention + deferred normalize + stage C ----
                with (
                    nc.named_scope("stageB"),
                    tc.tile_pool(name="exp", bufs=4) as exp_pool,
                    tc.tile_pool(name="expi", bufs=2) as expi_pool,
                    tc.tile_pool(name="small", bufs=2) as small,
                    tc.tile_pool(name="wop", bufs=1) as wop,
                    tc.tile_pool(name="sgp", bufs=1) as sgp,
                    tc.tile_pool(name="evC", bufs=4) as evC,
                    tc.tile_pool(name="agp", bufs=1) as agp,
                    tc.tile_pool(name="pssc", bufs=2, space="PSUM") as pssc,
                    tc.tile_pool(name="psat", bufs=2, space="PSUM") as psat,
                ):
                    IW = 2 * TCH               # 1024-wide i-half
                    wo_sb = wop.tile([P, 2, D], BF16)
                    nc.sync.dma_start(wo_sb[:], wo3)
                    attnG_sb = agp.tile([P, 2, T], BF16)
                    den = sgp.tile([P, 2, IW], F32R)  # ping-pong by ihalf parity
                    nc.vector.memset(den[0:HD, :, :].bitcast(F32), 0.0)

                    pending = []   # deferred normalize args

                    def emit_C(trange):
                        # C psum comes from the psat ring (flush() must have
                        # drained pending normalizes first)
                        for tt in trange:
                            tsl = slice(tt * P, (tt + 1) * P)
                            ev = evC.tile([P, D], BF16, tag="evC")
                            for oh in range(2):
                                ps = psat.tile([P, IW], F32, tag="psat")
                                for mc in range(2):
                                    for ii in range(2):
                                        o0 = oh * IW + ii * TCH
                                        nc.tensor.matmul(
                                            ps[:, ii * TCH:(ii + 1) * TCH],
                                            lhsT=attnG_sb[:, mc, tsl],
                                            rhs=wo_sb[:, mc, o0:o0 + TCH],
                                            start=(mc == 0),
                                            stop=(mc == 1),
                                        )
                                osl = slice(oh * IW, (oh + 1) * IW)
                                if oh == 0:
                                    nc.vector.tensor_copy(ev[:, osl], ps[:])
                                else:
                                    nc.scalar.copy(ev[:, osl], ps[:])
                            nc.sync.dma_start(out.ap()[tsl, :], ev[:])

                    def normalize(b, h, ih, a_ps):
                        # denom is psum row 64 (ones-col accumulation); rows
                        # 65:128 are zeros. Copy rows 64:128 aligned into den
                        # (rows 0:64 pre-zeroed once), broadcast row 64 to all
                        # partitions via the row-64 selector matmul, 1/x, then
                        # the gated muls (gate mul on the Pool engine).
                        hp = (h % 2) * HD
                        mo = h // 2
                        par = ih % 2
                        osl = slice(b * S + ih * IW, b * S + (ih + 1) * IW)
                        nc.vector.tensor_copy(den[HD:P, par, :], a_ps[HD:P, :])
                        bc_ps = pssc.tile([P, IW], F32, tag="pssc")
                        for ii in range(2):
                            nc.tensor.matmul(
                                bc_ps[:, ii * TCH:(ii + 1) * TCH],
                                lhsT=osel_sb[:],
                                rhs=den[:, par, ii * TCH:(ii + 1) * TCH],
                                start=True, stop=True,
                            )
                        rcp = small.tile([P, IW], F32, tag="rcp")
                        # reciprocal_approx_fast misbehaves at base partition
                        # 64; bc rows are all the denominator, so always use
                        # rows 0:64 (mixed-space mul allows base mismatch)
                        nc.vector.reciprocal_approx_fast(
                            out=rcp[0:HD, :], in_=bc_ps[0:HD, :])
                        ag = attnG_sb[hp:hp + HD, mo, osl]
                        nc.vector.tensor_mul(
                            out=ag, in0=a_ps[0:HD, :], in1=rcp[0:HD, :])
                        nc.gpsimd.tensor_mul(
                            out=ag, in0=ag, in1=gsb[hp:hp + HD, mo, osl])

                    def flush():
                        while pending:
                            normalize(*pending.pop(0))

                    for b in range(B):
                        for h in range(NH):
                            mo = h // 2
                            kvt = kv2lo if h % 2 == 0 else kv2hi
                            for ih in range(2):
                                i0 = b * S + ih * IW
                                isl = slice(i0, i0 + IW)
                                a_ps = psat.tile([P, IW], F32, tag="psat")

                                def scores_exp(jc):
                                    jsl = slice(b * S + jc * P,
                                                b * S + (jc + 1) * P)
                                    s_ps = pssc.tile([P, IW], F32, tag="pssc")
                                    for ii in range(2):
                                        nc.tensor.matmul(
                                            s_ps[:, ii * TCH:(ii + 1) * TCH],
                                            lhsT=kvt[:, jsl],
                                            rhs=qP_sb[:, mo,
                                                      i0 + ii * TCH:
                                                      i0 + (ii + 1) * TCH],
                                            start=True,
                                            stop=True,
                                        )
                                    if jc in DVE_JCS:
                                        yi = expi_pool.tile([P, IW], I32,
                                                            tag="expi")
                                        nc.vector.tensor_scalar(
                                            out=yi[:], in0=s_ps[:],
                                            scalar1=AS_CONST, scalar2=BS_CONST,
                                            op0=ALU.mult, op1=ALU.add,
                                        )
                                        return yi
                                    ex = exp_pool.tile([P, IW], BF16,
                                                       tag="exp")
                                    nc.scalar.activation(
                                        ex[:], s_ps[:], AF.Exp, scale=0.125)
                                    return ex

                                def attn_acc(jc, ex):
                                    if ex.dtype == I32:
                                        # top 16 bits of the Schraudolph int32
                                        # ARE the bf16 exp value; emit per
                                        # psum-bank halves so the strided AP
                                        # survives codegen
                                        for ii in range(2):
                                            r = ex[:, ii * TCH:(ii + 1) * TCH]
                                            r = r.bitcast(BF16).rearrange(
                                                "p (n t) -> p n t", t=2)[:, :, 1]
                                            nc.tensor.matmul(
                                                a_ps[:, ii * TCH:(ii + 1) * TCH],
                                                lhsT=v1_sb[:, b * SJ + jc, :],
                                                rhs=r,
                                                start=(jc == 0),
                                                stop=(jc == SJ - 1),
                                            )
                                        return
                                    for ii in range(2):
                                        nc.tensor.matmul(
                                            a_ps[:, ii * TCH:(ii + 1) * TCH],
                                            lhsT=v1_sb[:, b * SJ + jc, :],
                                            rhs=ex[:, ii * TCH:(ii + 1) * TCH],
                                            start=(jc == 0),
                                            stop=(jc == SJ - 1),
                                        )

                                prev = scores_exp(0)
                                for jc in range(1, SJ):
                                    cur = scores_exp(jc)
                                    if jc == 1:
                                        flush()
                                    attn_acc(jc - 1, prev)
                                    prev = cur
                                attn_acc(SJ - 1, prev)
                                pending.append((b, h, ih, a_ps))
                            # interleave prev batch's out-projection chunks;
                            # flush first so no pending normalize still needs
                            # a psat slot the C tiles will recycle
                            if b == 1:
                                flush()
                                emit_C(range(h * 4, (h + 1) * 4))
                    flush()
                    emit_C(range(NT // 2, NT))

    nc.compile()
    return nc


_NC_CACHE = None


def _get_nc(nreps=1):
    global _NC_CACHE
    if _NC_CACHE is None:
        _NC_CACHE = {}
    if nreps not in _NC_CACHE:
        _NC_CACHE[nreps] = build_nc(nreps)
    return _NC_CACHE[nreps]


def _dup_rows(tab64):
    """[64, S] -> [128, S] with both partition halves holding the table."""
    return np.ascontiguousarray(np.concatenate([tab64, tab64], axis=0))


def _prep_inputs(hidden_states, cos, sin, Wq, Wk, Wv, Wo):
    hs = np.asarray(hidden_states, dtype=np.float32)
    cos = np.asarray(cos, dtype=np.float32)
    sin = np.asarray(sin, dtype=np.float32)
    Wq = np.asarray(Wq, dtype=np.float32)
    Wk = np.asarray(Wk, dtype=np.float32)
    Wv = np.asarray(Wv, dtype=np.float32)
    Wo = np.asarray(Wo, dtype=np.float32)

    bf16 = mybir.dt.np(BF16)
    hsT = np.ascontiguousarray(hs.reshape(T, D).T).astype(bf16)

    cosT = cos.T                                     # [64, S]
    sinT = sin.T
    sin_signed = np.concatenate([-sinT[:HH], sinT[HH:]], axis=0)
    osel = np.zeros((P, P), np.float32)
    osel[HD, :] = 1.0
    prot = np.zeros((P, P), np.float32)
    for k in range(P):
        prot[k, k ^ HH] = 1.0
    common = {
        "hsT": hsT,
        "ck": _dup_rows(cosT),
        "sk": _dup_rows(sin_signed),
        "ident": np.eye(HD, dtype=np.float32),
        "osel": osel,
        "prot": prot,
        "ones": np.ones((P, B * SJ), mybir.dt.np(BF16)),
    }
    in_maps = []
    for c in range(NCORES):
        qcols = Wq[:, c * MQ:(c + 1) * MQ]
        gcols = Wq[:, H * HD + c * MQ: H * HD + (c + 1) * MQ]
        in_maps.append(
            {
                **common,
                "wqg": np.ascontiguousarray(
                    np.concatenate([qcols, gcols], axis=1)
                ).astype(bf16),
                "wkv": np.ascontiguousarray(
                    np.concatenate(
                        [Wk[:, c * HD:(c + 1) * HD], Wv[:, c * HD:(c + 1) * HD]],
                        axis=1,
                    )
                ).astype(bf16),
                "wo": np.ascontiguousarray(Wo[c * MQ:(c + 1) * MQ, :]).astype(bf16),
            }
        )
    return in_maps


def kernel(hidden_states, cos, sin, Wq, Wk, Wv, Wo, _trace=False, _trace_kwargs=None):
    nc = _get_nc()
    in_maps = _prep_inputs(hidden_states, cos, sin, Wq, Wk, Wv, Wo)
    res = run_bass_kernel_spmd(
        nc, in_maps, list(range(NCORES)), trace=_trace, **(_trace_kwargs or {})
    )
    total = res.results[0]["out"].astype(np.float32).copy()
    for c in range(1, NCORES):
        total += res.results[c]["out"]
    out = total.reshape(B, S, D)
    if _trace:
        kernel._last_results = res
    return out



# revision 11
# speedup vs baseline: 1.0988x; 1.0988x over previous
"""Tensor-parallel GQA attention (sigmoid-gated) for Trainium2, 8 NeuronCores.

Problem: B=2, S=2048, D=2048, H=32 q-heads, KV=8 kv-heads, HD=64 (GQA groups=4),
RoPE on q/k, full (non-causal) softmax, sigmoid(gate) output gating, out proj.

Sharding (tensor-parallel over heads): core c owns q-heads 4c..4c+3, kv-head c,
the matching 256 q-cols + 256 gate-cols of Wq, 64-col slices of Wk/Wv, and rows
256c:256c+256 of Wo. Each core computes a full [B*S, D] partial of the output
projection; the host sums the 8 partials.

Per-core pipeline (projection/attn-prob/out-proj matmuls in bf16, scores in
float32r; rel-L2 error ~1e-2 vs the 2e-2 gate):
  A) projections psum[m,t] += W[d,m].T @ hsT[d,t], one 2MB hsT DMA per
     512-column chunk (DMA-issue cadence is a bottleneck, not bandwidth).
     q lands in head-PAIR layout qP[128, 2, T] (head 2m at partitions 0:64,
     head 2m+1 at 64:128); k is evicted to kv2lo (rows 0:64, zeros above) and
     mirrored into kv2hi (rows 64:128, zeros below) so each head's scores
     contract K=128 against the zero-padded copy matching its partition half.
     RoPE is fused per chunk: rotate-half runs as a PE permutation matmul
     into PSUM (walrus forbids partition-shifted SBUF+SBUF TensorTensor),
     then three partition-aligned DVE ops finish x*cos + rot*sin.  Gate
     columns get sigmoid at eviction into a resident bf16 SBUF tile.
  B) attention in scoresT orientation over 1024-wide i-halves:
     scoresT[j,i] = kv[:,j].T @ qP[:,i] (two bank-aligned 512 psum writes),
     probs = exp(s/8) via ACT activation scale=0.125 into bf16 for most
     j-chunks and via a 1-op DVE Schraudolph bit-trick (int32 tensor_scalar;
     the top 16 bits ARE the bf16 value, consumed through a stride-2 bf16
     view) for DVE_JCS chunks -- splits the activation load across engines;
     softmax renormalization absorbs most of the approximation error.
     attnT[hd,i] += v1[j, hd|1].T @ probs in bf16; v1's ones-column
     accumulates denominators in psum row 64 for free.  Normalization is
     deferred one i-half (runs under the next one's matmuls): denom row ->
     SBUF, broadcast via a row-64-selector matmul, reciprocal_approx_fast
     (rows 0:64 only -- the custom op misbehaves at base partition 64),
     then a gated mul on DVE and the sigmoid-gate mul on the Pool engine.
  C) out[t,dout] += attnG[m,t].T @ Wo[m,dout] bf16 partials written as bf16,
     sharing the psat psum ring, interleaved per 4-t-tile chunks between
     batch-1 heads so eviction load spreads.
"""

import sys

sys.path.insert(0, "/opt/trn_rl_repo")

import numpy as np

import concourse.bass as bass  # noqa: F401
import concourse.mybir as mybir
import concourse.tile as tile
from concourse import bacc
from concourse.bass_utils import run_bass_kernel_spmd

F32 = mybir.dt.float32
F32R = mybir.dt.float32r
I32 = mybir.dt.int32
BF16 = mybir.dt.bfloat16
AF = mybir.ActivationFunctionType
ALU = mybir.AluOpType

P = 128
B, S, D = 2, 2048, 2048
T = B * S                  # 4096 token rows (batch folded)
H, KV, HD = 32, 8, 64
HH = HD // 2               # 32
NCORES = 8
NH = H // NCORES           # 4 q-heads per core
MQ = NH * HD               # 256 q-cols per core
DC = D // P                # 16 contraction chunks
TCH = 512                  # moving-dim chunk
NTCH = T // TCH            # 8
SJ = S // P                # 16 key chunks per batch
NSEG = S // TCH            # 4 i-segments per batch
NT = T // P                # 32 t-tiles
NP = SJ // 2               # 8 j-pairs per segment

# Schraudolph fast-exp (DVE): exp(s/8) ~= bitcast_i32_f32(s*AS + BS)
AS_CONST = float((1 << 23) * 1.4426950408889634 * 0.125)
BS_CONST = float(127 * (1 << 23) - 449200)
# which j-chunks (of 16 per i-half) compute probs on DVE instead of ACT.
# Exp runs per 512-half (chain latency ~1us -> ~0.6us; 4 single-bank psum
# slots pipeline 2 full jc ahead), so PE (scores+attn+C ~17.5us/ihalf in
# the C-burst windows) paces stage B; ACT ~= 10 exp + den copy + C-evict,
# DVE ~= 6 exp + 2 recip + norm-mul + C-evict, both <= ~15us/ihalf.
DVE_JCS = (2, 5, 8, 11, 13, 15)


def build_nc(nreps=1):
    nc = bacc.Bacc("TRN2", target_bir_lowering=False, debug=False)

    hsT = nc.dram_tensor("hsT", [D, T], BF16, kind="ExternalInput")
    wqg = nc.dram_tensor("wqg", [D, 2 * MQ], BF16, kind="ExternalInput")
    wkv = nc.dram_tensor("wkv", [D, 2 * HD], BF16, kind="ExternalInput")
    wo = nc.dram_tensor("wo", [MQ, D], BF16, kind="ExternalInput")
    # rope tables, [128, S] with rows duplicated (row p holds entry p % 64)
    ck = nc.dram_tensor("ck", [P, S], F32, kind="ExternalInput")   # cos
    sk = nc.dram_tensor("sk", [P, S], F32, kind="ExternalInput")   # signed sin
    identd = nc.dram_tensor("ident", [HD, HD], F32R, kind="ExternalInput")
    oseld = nc.dram_tensor("osel", [P, P], F32R, kind="ExternalInput")  # row64=1
    protd = nc.dram_tensor("prot", [P, P], F32R, kind="ExternalInput")  # xor-32 perm
    onesd = nc.dram_tensor("ones", [P, B * SJ], BF16, kind="ExternalInput")
    out = nc.dram_tensor("out", [T, D], BF16, kind="ExternalOutput")

    hsT3 = hsT.ap().rearrange("(o p) t -> p o t", p=P)   # [128, 16, 4096]
    wqg3 = wqg.ap().rearrange("(o p) m -> p o m", p=P)   # [128, 16, 512]
    wkv3 = wkv.ap().rearrange("(o p) m -> p o m", p=P)   # [128, 16, 128]
    wo3 = wo.ap().rearrange("(o p) n -> p o n", p=P)     # [128, 2, 2048]

    with tile.TileContext(nc) as tc:
        for _rep in range(nreps):
            with (
                tc.tile_pool(name="const", bufs=1) as const,
                tc.tile_pool(name="big", bufs=1) as big,
            ):
                ident_sb = const.tile([HD, HD], F32R)
                osel_sb = const.tile([P, P], F32R)  # row 64 ones, rest zero
                prot_sb = const.tile([P, P], F32R)  # xor-32 permutation
                nc.sync.dma_start(ident_sb[:], identd.ap())
                nc.sync.dma_start(osel_sb[:], oseld.ap())
                nc.sync.dma_start(prot_sb[:], protd.ap())

                # ---- persistent activations ----
                qP_sb = big.tile([P, 2, T], F32R)    # head pairs, roped in place
                kv2lo = big.tile([P, T], F32R)       # roped k rows 0:64, 0 above
                kv2hi = big.tile([P, T], F32R)       # 0 below, roped k rows 64:128
                v1_sb = big.tile([P, B * SJ, P], BF16)  # v | ones col | zeros
                gsb = big.tile([P, 2, T], BF16)      # sigmoid(gate), resident

                # ---- stage A: projections + fused rope ----
                with (
                    nc.named_scope("stageA"),
                    tc.tile_pool(name="wpool", bufs=1) as wpool,
                    tc.tile_pool(name="tab", bufs=1) as tab,
                    tc.tile_pool(name="hst", bufs=3) as hst_pool,
                    tc.tile_pool(name="vst", bufs=8) as vst,
                    tc.tile_pool(name="ps512", bufs=5, space="PSUM") as ps512,
                    tc.tile_pool(name="psrot", bufs=3, space="PSUM") as psrot,
                ):
                    ck_sb = tab.tile([P, S], F32)
                    sk_sb = tab.tile([P, S], F32)

                    wqg_sb = wpool.tile([P, DC, 2 * MQ], BF16)
                    wkv_sb = wpool.tile([P, DC, 2 * HD], BF16)
                    # zero pads: scores contract K=128 with a zeroed half so
                    # each head pair partition-half stays independent
                    nc.gpsimd.memset(kv2lo[HD:P, :].bitcast(F32), 0.0)
                    nc.gpsimd.memset(kv2hi[0:HD, :].bitcast(F32), 0.0)
                    # v1 cols 65:128 zero -> psum rows 65:128 stay finite zeros
                    nc.gpsimd.memset(v1_sb[:, :, HD + 1:P], 0.0)

                    def rope(x, tabsl, rows):
                        # x: [128, TCH] slice, roped in place on rows [0:rows].
                        # rot_half via PE permutation matmul (psum), then only
                        # partition-aligned TTs: rot*=sin; x*=cos; x+=rot.
                        rot = psrot.tile([P, TCH], F32, tag="rot")
                        nc.tensor.matmul(
                            rot[:], lhsT=prot_sb[:], rhs=x, start=True, stop=True)
                        c = ck_sb[:, tabsl]
                        s = sk_sb[:, tabsl]
                        xr = x[0:rows, :] if rows < P else x
                        nc.vector.tensor_mul(
                            out=rot[0:rows, :], in0=rot[0:rows, :],
                            in1=s[0:rows, :])
                        nc.vector.tensor_mul(out=xr, in0=xr, in1=c[0:rows, :])
                        nc.vector.tensor_add(out=xr, in0=xr, in1=rot[0:rows, :])

                    vstgs = []
                    pend_rope = []
                    for tci in range(NTCH):
                        ts = slice(tci * TCH, (tci + 1) * TCH)
                        t0 = (tci * TCH) % S
                        tabsl = slice(t0, t0 + TCH)
                        if tci == 0:
                            nc.sync.dma_start(wkv_sb[:], wkv3)
                            for dq in range(4):
                                nc.sync.dma_start(
                                    wqg_sb[:, dq * 4:(dq + 1) * 4, :],
                                    wqg3[:, dq * 4:(dq + 1) * 4, :])
                        ht = hst_pool.tile([P, DC * TCH], BF16, tag="hst")
                        hview = ht[:].rearrange("p (o t) -> p o t", o=DC)
                        if tci == 0:
                            for db in range(4):
                                nc.sync.dma_start(
                                    hview[:, db * 4:(db + 1) * 4, :],
                                    hsT3[:, db * 4:(db + 1) * 4, ts])
                        else:
                            nc.sync.dma_start(hview, hsT3[:, :, ts])
                        if tci == 0:
                            nc.sync.dma_start(ck_sb[:], ck.ap())
                            nc.sync.dma_start(sk_sb[:], sk.ap())
                        pss = [ps512.tile([P, TCH], F32, tag="ps512",
                                          name=f"psA{_m}") for _m in range(5)]
                        for dc in range(DC):
                            for mt in range(5):  # 0: kv, 1-2: q pairs, 3-4: gate
                                if mt == 0:
                                    w = wkv_sb[:, dc, :]
                                else:
                                    w = wqg_sb[:, dc, (mt - 1) * P:mt * P]
                                nc.tensor.matmul(
                                    pss[mt][:],
                                    lhsT=w,
                                    rhs=ht[:, dc * TCH:(dc + 1) * TCH],
                                    start=(dc == 0),
                                    stop=(dc == DC - 1),
                                )
                        if pend_rope:
                            pend_rope.pop(0)()
                        # evictions
                        nc.vector.tensor_copy(kv2lo[0:HD, ts], pss[0][0:HD, :])
                        vstg = vst.tile([HD, TCH], F32R, tag="vst")
                        nc.vector.tensor_copy(vstg[:], pss[0][HD:P, :])
                        nc.scalar.copy(qP_sb[:, 0, ts], pss[1][:])
                        nc.scalar.copy(qP_sb[:, 1, ts], pss[2][:])
                        for mo in range(2):
                            nc.scalar.activation(gsb[:, mo, ts], pss[3 + mo][:],
                                                 AF.Sigmoid)
                        # rope deferred one chunk: the rot matmuls land
                        # in the PE queue after the NEXT chunk's projection
                        # matmuls, whose evictions are then long done
                        def do_rope(ts=ts, tabsl=tabsl):
                            rope(kv2lo[:, ts], tabsl, HD)
                            nc.vector.tensor_copy(kv2hi[HD:P, ts],
                                                  kv2lo[0:HD, ts])
                            rope(qP_sb[:, 0, ts], tabsl, P)
                            rope(qP_sb[:, 1, ts], tabsl, P)
                        pend_rope.append(do_rope)
                        vstgs.append(vstg)
                    while pend_rope:
                        pend_rope.pop(0)()
                    # batched v transposes: PE transpose-mode entered once
                    for tci in range(NTCH):
                        for j4 in range(TCH // P):
                            jc = tci * (TCH // P) + j4
                            vt_ps = psrot.tile([P, HD], F32R, tag="rot")
                            nc.tensor.transpose(
                                vt_ps[:],
                                vstgs[tci][:, j4 * P:(j4 + 1) * P],
                                ident_sb[:],
                            )
                            nc.scalar.copy(v1_sb[:, jc, 0:HD], vt_ps[:])
                    nc.sync.dma_start(v1_sb[:, :, HD:HD + 1], onesd.ap()[:, :, None])

                # ---- stage B: attention + deferred normalize + stage C ----
                with (
                    nc.named_scope("stageB"),
                    tc.tile_pool(name="exp", bufs=4) as exp_pool,
                    tc.tile_pool(name="expi", bufs=3) as expi_pool,
                    tc.tile_pool(name="small", bufs=2) as small,
                    tc.tile_pool(name="wop", bufs=1) as wop,
                    tc.tile_pool(name="sgp", bufs=1) as sgp,
                    tc.tile_pool(name="evC", bufs=4) as evC,
                    tc.tile_pool(name="agp", bufs=1) as agp,
                    tc.tile_pool(name="pssc", bufs=4, space="PSUM") as pssc,
                    tc.tile_pool(name="psat", bufs=2, space="PSUM") as psat,
                ):
                    IW = 2 * TCH               # 1024-wide i-half
                    wo_sb = wop.tile([P, 2, D], BF16)
                    nc.sync.dma_start(wo_sb[:], wo3)
                    attnG_sb = agp.tile([P, 2, T], BF16)
                    den = sgp.tile([P, 2, IW], F32R)  # ping-pong by ihalf parity
                    nc.vector.memset(den[0:HD, :, :].bitcast(F32), 0.0)

                    pending = []   # deferred normalize args

                    def emit_C(trange):
                        # C psum comes from the 512-wide pssc ring (no psat
                        # conflict with in-flight deferred normalizes)
                        for tt in trange:
                            tsl = slice(tt * P, (tt + 1) * P)
                            ev = evC.tile([P, D], BF16, tag="evC")
                            for oh in range(2):
                                for ii in range(2):
                                    o0 = oh * IW + ii * TCH
                                    ps = pssc.tile([P, TCH], F32, tag="pssc")
                                    for mc in range(2):
                                        nc.tensor.matmul(
                                            ps[:],
                                            lhsT=attnG_sb[:, mc, tsl],
                                            rhs=wo_sb[:, mc, o0:o0 + TCH],
                                            start=(mc == 0),
                                            stop=(mc == 1),
                                        )
                                    osl = slice(o0, o0 + TCH)
                                    if oh == 0:
                                        nc.vector.tensor_copy(ev[:, osl], ps[:])
                                    else:
                                        nc.scalar.copy(ev[:, osl], ps[:])
                            nc.sync.dma_start(out.ap()[tsl, :], ev[:])

                    def emit_den(ih, a_ps):
                        # denom is psum row 64 (ones-col accumulation); rows
                        # 65:128 are zeros. Copy rows 64:128 aligned into den
                        # (rows 0:64 pre-zeroed once) right at i-half end:
                        # DVE is idle at the boundary (its first exp chunk is
                        # jc=2), so this never blocks the next i-half's exps.
                        par = ih % 2
                        for ii in range(2):
                            isl = slice(ii * TCH, (ii + 1) * TCH)
                            nc.vector.tensor_copy(den[HD:P, par, isl],
                                                  a_ps[HD:P, isl])

                    def normalize(b, h, ih, a_ps):
                        # broadcast row 64 to all partitions via the row-64
                        # selector matmul, 1/x, then the gated muls (gate mul
                        # on the Pool engine). Runs deferred mid-next-ihalf
                        # so it doesn't collide with the boundary exps.
                        hp = (h % 2) * HD
                        mo = h // 2
                        par = ih % 2
                        osl = slice(b * S + ih * IW, b * S + (ih + 1) * IW)
                        rcp = small.tile([P, IW], F32, tag="rcp")
                        for ii in range(2):
                            isl = slice(ii * TCH, (ii + 1) * TCH)
                            bc_ps = pssc.tile([P, TCH], F32, tag="pssc")
                            nc.tensor.matmul(
                                bc_ps[:],
                                lhsT=osel_sb[:],
                                rhs=den[:, par, isl],
                                start=True, stop=True,
                            )
                            # reciprocal_approx_fast misbehaves at base
                            # partition 64; bc rows are all the denominator,
                            # so always use rows 0:64 (mixed-space mul allows
                            # base mismatch)
                            nc.vector.reciprocal_approx_fast(
                                out=rcp[0:HD, isl], in_=bc_ps[0:HD, :])
                        ag = attnG_sb[hp:hp + HD, mo, osl]
                        nc.vector.tensor_mul(
                            out=ag, in0=a_ps[0:HD, :], in1=rcp[0:HD, :])
                        nc.gpsimd.tensor_mul(
                            out=ag, in0=ag, in1=gsb[hp:hp + HD, mo, osl])

                    def flush():
                        while pending:
                            normalize(*pending.pop(0))

                    for b in range(B):
                        for h in range(NH):
                            mo = h // 2
                            kvt = kv2lo if h % 2 == 0 else kv2hi
                            for ih in range(2):
                                i0 = b * S + ih * IW
                                isl = slice(i0, i0 + IW)
                                a_ps = psat.tile([P, IW], F32, tag="psat")

                                def scores_exp(jc):
                                    jsl = slice(b * S + jc * P,
                                                b * S + (jc + 1) * P)
                                    dve = jc in DVE_JCS
                                    if dve:
                                        ex = expi_pool.tile([P, IW], I32,
                                                            tag="expi")
                                    else:
                                        ex = exp_pool.tile([P, IW], BF16,
                                                           tag="exp")
                                    for ii in range(2):
                                        isl = slice(ii * TCH, (ii + 1) * TCH)
                                        s_ps = pssc.tile([P, TCH], F32,
                                                         tag="pssc")
                                        nc.tensor.matmul(
                                            s_ps[:],
                                            lhsT=kvt[:, jsl],
                                            rhs=qP_sb[:, mo,
                                                      i0 + ii * TCH:
                                                      i0 + (ii + 1) * TCH],
                                            start=True,
                                            stop=True,
                                        )
                                        if dve:
                                            nc.vector.tensor_scalar(
                                                out=ex[:, isl], in0=s_ps[:],
                                                scalar1=AS_CONST,
                                                scalar2=BS_CONST,
                                                op0=ALU.mult, op1=ALU.add,
                                            )
                                        else:
                                            nc.scalar.activation(
                                                ex[:, isl], s_ps[:],
                                                AF.Exp, scale=0.125)
                                    return ex

                                def attn_acc(jc, ex):
                                    if ex.dtype == I32:
                                        # top 16 bits of the Schraudolph int32
                                        # ARE the bf16 exp value; emit per
                                        # psum-bank halves so the strided AP
                                        # survives codegen
                                        for ii in range(2):
                                            r = ex[:, ii * TCH:(ii + 1) * TCH]
                                            r = r.bitcast(BF16).rearrange(
                                                "p (n t) -> p n t", t=2)[:, :, 1]
                                            nc.tensor.matmul(
                                                a_ps[:, ii * TCH:(ii + 1) * TCH],
                                                lhsT=v1_sb[:, b * SJ + jc, :],
                                                rhs=r,
                                                start=(jc == 0),
                                                stop=(jc == SJ - 1),
                                            )
                                        return
                                    for ii in range(2):
                                        nc.tensor.matmul(
                                            a_ps[:, ii * TCH:(ii + 1) * TCH],
                                            lhsT=v1_sb[:, b * SJ + jc, :],
                                            rhs=ex[:, ii * TCH:(ii + 1) * TCH],
                                            start=(jc == 0),
                                            stop=(jc == SJ - 1),
                                        )

                                prev = scores_exp(0)
                                for jc in range(1, SJ):
                                    cur = scores_exp(jc)
                                    if jc == 6:
                                        flush()
                                    attn_acc(jc - 1, prev)
                                    prev = cur
                                attn_acc(SJ - 1, prev)
                                emit_den(ih, a_ps)
                                pending.append((b, h, ih, a_ps))
                            # interleave prev batch's out-projection chunks
                            # (C runs on the pssc ring, no psat conflict with
                            # the still-pending normalize)
                            if b == 1:
                                emit_C(range(h * 4, (h + 1) * 4))
                    flush()
                    emit_C(range(NT // 2, NT))

    nc.compile()
    return nc


_NC_CACHE = None


def _get_nc(nreps=1):
    global _NC_CACHE
    if _NC_CACHE is None:
        _NC_CACHE = {}
    if nreps not in _NC_CACHE:
        _NC_CACHE[nreps] = build_nc(nreps)
    return _NC_CACHE[nreps]


def _dup_rows(tab64):
    """[64, S] -> [128, S] with both partition halves holding the table."""
    return np.ascontiguousarray(np.concatenate([tab64, tab64], axis=0))


def _prep_inputs(hidden_states, cos, sin, Wq, Wk, Wv, Wo):
    hs = np.asarray(hidden_states, dtype=np.float32)
    cos = np.asarray(cos, dtype=np.float32)
    sin = np.asarray(sin, dtype=np.float32)
    Wq = np.asarray(Wq, dtype=np.float32)
    Wk = np.asarray(Wk, dtype=np.float32)
    Wv = np.asarray(Wv, dtype=np.float32)
    Wo = np.asarray(Wo, dtype=np.float32)

    bf16 = mybir.dt.np(BF16)
    hsT = np.ascontiguousarray(hs.reshape(T, D).T).astype(bf16)

    cosT = cos.T                                     # [64, S]
    sinT = sin.T
    sin_signed = np.concatenate([-sinT[:HH], sinT[HH:]], axis=0)
    osel = np.zeros((P, P), np.float32)
    osel[HD, :] = 1.0
    prot = np.zeros((P, P), np.float32)
    for k in range(P):
        prot[k, k ^ HH] = 1.0
    common = {
        "hsT": hsT,
        "ck": _dup_rows(cosT),
        "sk": _dup_rows(sin_signed),
        "ident": np.eye(HD, dtype=np.float32),
        "osel": osel,
        "prot": prot,
        "ones": np.ones((P, B * SJ), mybir.dt.np(BF16)),
    }
    in_maps = []
    for c in range(NCORES):
        qcols = Wq[:, c * MQ:(c + 1) * MQ]
        gcols = Wq[:, H * HD + c * MQ: H * HD + (c + 1) * MQ]
        in_maps.append(
            {
                **common,
                "wqg": np.ascontiguousarray(
                    np.concatenate([qcols, gcols], axis=1)
                ).astype(bf16),
                "wkv": np.ascontiguousarray(
                    np.concatenate(
                        [Wk[:, c * HD:(c + 1) * HD], Wv[:, c * HD:(c + 1) * HD]],
                        axis=1,
                    )
                ).astype(bf16),
                "wo": np.ascontiguousarray(Wo[c * MQ:(c + 1) * MQ, :]).astype(bf16),
            }
        )
    return in_maps


def kernel(hidden_states, cos, sin, Wq, Wk, Wv, Wo, _trace=False, _trace_kwargs=None):
    nc = _get_nc()
    in_maps = _prep_inputs(hidden_states, cos, sin, Wq, Wk, Wv, Wo)
    res = run_bass_kernel_spmd(
        nc, in_maps, list(range(NCORES)), trace=_trace, **(_trace_kwargs or {})
    )
    total = res.results[0]["out"].astype(np.float32).copy()
    for c in range(1, NCORES):
        total += res.results[c]["out"]
    out = total.reshape(B, S, D)
    if _trace:
        kernel._last_results = res
    return out



# revision 38
# speedup vs baseline: 1.1400x; 1.0375x over previous
"""Tensor-parallel GQA attention (sigmoid-gated) for Trainium2, 8 NeuronCores.

Problem: B=2, S=2048, D=2048, H=32 q-heads, KV=8 kv-heads, HD=64 (GQA groups=4),
RoPE on q/k, full (non-causal) softmax, sigmoid(gate) output gating, out proj.

Sharding (tensor-parallel over heads): core c owns q-heads 4c..4c+3, kv-head c,
the matching 256 q-cols + 256 gate-cols of Wq, 64-col slices of Wk/Wv, and rows
256c:256c+256 of Wo. Each core computes a full [B*S, D] partial of the output
projection; the host sums the 8 partials.

Per-core pipeline (projection/attn-prob/out-proj matmuls in bf16, scores in
float32r; rel-L2 error ~1e-2 vs the 2e-2 gate):
  A) projections psum[m,t] += W[d,m].T @ hsT[d,t], one 2MB hsT DMA per
     512-column chunk (DMA-issue cadence is a bottleneck, not bandwidth).
     q lands in head-PAIR layout qP[128, 2, T] (head 2m at partitions 0:64,
     head 2m+1 at 64:128); k is evicted to kv2lo (rows 0:64, zeros above) and
     mirrored into kv2hi (rows 64:128, zeros below) so each head's scores
     contract K=128 against the zero-padded copy matching its partition half.
     RoPE is fused per chunk: rotate-half runs as a PE permutation matmul
     into PSUM (walrus forbids partition-shifted SBUF+SBUF TensorTensor),
     then three partition-aligned DVE ops finish x*cos + rot*sin.  Gate
     columns get sigmoid at eviction into a resident bf16 SBUF tile.
  B) attention in scoresT orientation over 1024-wide i-halves:
     scoresT[j,i] = kv[:,j].T @ qP[:,i] (two bank-aligned 512 psum writes),
     probs = exp(s/8) via ACT activation scale=0.125 into bf16 for most
     j-chunks and via a 1-op DVE Schraudolph bit-trick (int32 tensor_scalar;
     the top 16 bits ARE the bf16 value, consumed through a stride-2 bf16
     view) for DVE_JCS chunks -- splits the activation load across engines;
     softmax renormalization absorbs most of the approximation error.
     attnT[hd,i] += v1[j, hd|1].T @ probs in bf16; v1's ones-column
     accumulates denominators in psum row 64 for free.  Normalization is
     deferred one i-half (runs under the next one's matmuls): denom row ->
     SBUF, broadcast via a row-64-selector matmul, reciprocal_approx_fast
     (rows 0:64 only -- the custom op misbehaves at base partition 64),
     then a gated mul on DVE and the sigmoid-gate mul on the Pool engine.
  C) out[t,dout] += attnG[m,t].T @ Wo[m,dout] bf16 partials written as bf16,
     sharing the psat psum ring, interleaved per 4-t-tile chunks between
     batch-1 heads so eviction load spreads.
"""

import sys

sys.path.insert(0, "/opt/trn_rl_repo")

import numpy as np

import concourse.bass as bass  # noqa: F401
import concourse.mybir as mybir
import concourse.tile as tile
from concourse import bacc
from concourse.bass_utils import run_bass_kernel_spmd

F32 = mybir.dt.float32
F32R = mybir.dt.float32r
I32 = mybir.dt.int32
BF16 = mybir.dt.bfloat16
AF = mybir.ActivationFunctionType
ALU = mybir.AluOpType

P = 128
B, S, D = 2, 2048, 2048
T = B * S                  # 4096 token rows (batch folded)
H, KV, HD = 32, 8, 64
HH = HD // 2               # 32
NCORES = 8
NH = H // NCORES           # 4 q-heads per core
MQ = NH * HD               # 256 q-cols per core
DC = D // P                # 16 contraction chunks
TCH = 512                  # moving-dim chunk
NTCH = T // TCH            # 8
SJ = S // P                # 16 key chunks per batch
NSEG = S // TCH            # 4 i-segments per batch
NT = T // P                # 32 t-tiles
NP = SJ // 2               # 8 j-pairs per segment

# Schraudolph fast-exp (DVE): exp(s/8) ~= bitcast_i32_f32(s*AS + BS)
AS_CONST = float((1 << 23) * 1.4426950408889634 * 0.125)
BS_CONST = float(127 * (1 << 23) - 449200)
# which j-chunks (of 16 per i-half) compute probs on DVE instead of ACT.
# Exp runs per 512-half (chain latency ~1us -> ~0.6us; 4 single-bank psum
# slots pipeline 2 full jc ahead), so PE (scores+attn+C ~17.5us/ihalf in
# the C-burst windows) paces stage B; ACT ~= 10 exp + den copy + C-evict,
# DVE ~= 6 exp + 2 recip + norm-mul + C-evict, both <= ~15us/ihalf.
DVE_JCS = (2, 5, 8, 11, 13, 15)


def build_nc(nreps=1):
    nc = bacc.Bacc("TRN2", target_bir_lowering=False, debug=False)

    hsT = nc.dram_tensor("hsT", [D, T], BF16, kind="ExternalInput")
    wqg = nc.dram_tensor("wqg", [D, 2 * MQ], BF16, kind="ExternalInput")
    wkv = nc.dram_tensor("wkv", [D, 2 * HD], BF16, kind="ExternalInput")
    wo = nc.dram_tensor("wo", [MQ, D], BF16, kind="ExternalInput")
    # rope tables, [128, S] with rows duplicated (row p holds entry p % 64)
    ck = nc.dram_tensor("ck", [P, S], BF16, kind="ExternalInput")  # cos
    sk = nc.dram_tensor("sk", [P, S], BF16, kind="ExternalInput")  # signed sin
    identd = nc.dram_tensor("ident", [HD, HD], F32R, kind="ExternalInput")
    oseld = nc.dram_tensor("osel", [P, P], F32R, kind="ExternalInput")  # row64=1
    protd = nc.dram_tensor("prot", [P, P], F32R, kind="ExternalInput")  # xor-32 perm
    onesd = nc.dram_tensor("ones", [P, B * SJ], BF16, kind="ExternalInput")
    out = nc.dram_tensor("out", [T, D], BF16, kind="ExternalOutput")

    hsT3 = hsT.ap().rearrange("(o p) t -> p o t", p=P)   # [128, 16, 4096]
    wqg3 = wqg.ap().rearrange("(o p) m -> p o m", p=P)   # [128, 16, 512]
    wkv3 = wkv.ap().rearrange("(o p) m -> p o m", p=P)   # [128, 16, 128]
    wo3 = wo.ap().rearrange("(o p) n -> p o n", p=P)     # [128, 2, 2048]

    with tile.TileContext(nc) as tc:
        for _rep in range(nreps):
            with (
                tc.tile_pool(name="const", bufs=1) as const,
                tc.tile_pool(name="big", bufs=1) as big,
            ):
                ident_sb = const.tile([HD, HD], F32R)
                osel_sb = const.tile([P, P], F32R)  # row 64 ones, rest zero
                prot_sb = const.tile([P, P], F32R)  # xor-32 permutation
                ck_sb = const.tile([P, S], BF16)    # cos table (rope)
                sk_sb = const.tile([P, S], BF16)    # signed sin table

                # ---- persistent activations ----
                qP_sb = big.tile([P, 2, T], F32R)    # head pairs, roped in place
                kv2lo = big.tile([P, T], F32R)       # roped k rows 0:64, 0 above
                kv2hi = big.tile([P, T], F32R)       # 0 below, roped k rows 64:128
                v1_sb = big.tile([P, B * SJ, P], BF16)  # v | ones col | zeros
                gsb = big.tile([P, 2, T], BF16)      # sigmoid(gate), resident

                # ---- stage A: projections + fused rope ----
                with (
                    nc.named_scope("stageA"),
                    tc.tile_pool(name="wpool", bufs=1) as wpool,
                    tc.tile_pool(name="hst", bufs=3) as hst_pool,
                    tc.tile_pool(name="vst", bufs=8) as vst,
                    tc.tile_pool(name="ps512", bufs=5, space="PSUM") as ps512,
                    tc.tile_pool(name="psrot", bufs=2, space="PSUM") as psrot,
                    tc.tile_pool(name="psvt", bufs=1, space="PSUM") as psvt,
                ):
                    wqg_sb = wpool.tile([P, DC, 2 * MQ], BF16)
                    wkv_sb = wpool.tile([P, DC, 2 * HD], BF16)

                    def rope(x, tabsl, rows, pool, tag):
                        # x: [128, TCH] slice, roped in place on rows [0:rows].
                        # rot_half via PE permutation matmul (psum), then only
                        # partition-aligned TTs: rot*=sin; x*=cos; x+=rot.
                        rot = pool.tile([P, TCH], F32, tag=tag)
                        nc.tensor.matmul(
                            rot[:], lhsT=prot_sb[:], rhs=x, start=True, stop=True)
                        c = ck_sb[:, tabsl]
                        s = sk_sb[:, tabsl]
                        xr = x[0:rows, :] if rows < P else x
                        nc.vector.tensor_mul(
                            out=rot[0:rows, :], in0=rot[0:rows, :],
                            in1=s[0:rows, :])
                        nc.vector.tensor_mul(out=xr, in0=xr, in1=c[0:rows, :])
                        nc.vector.tensor_add(out=xr, in0=xr, in1=rot[0:rows, :])

                    vstgs = []
                    pend_rope = []
                    pend_vt = []

                    def do_vt(tci):
                        # transpose this chunk's v into v1 (per-chunk so the
                        # A->B junction never waits on a batched drain); one
                        # strided ACT copy evicts all 4 j-tiles at once
                        vt_ps = psvt.tile([P, TCH // P, HD], F32R, tag="vt")
                        for j4 in range(TCH // P):
                            nc.tensor.transpose(
                                vt_ps[:, j4, :],
                                vstgs[tci][:, j4 * P:(j4 + 1) * P],
                                ident_sb[:],
                            )
                        jc0 = tci * (TCH // P)
                        nc.scalar.copy(v1_sb[:, jc0:jc0 + TCH // P, 0:HD],
                                       vt_ps[:])

                    for tci in range(NTCH):
                        ts = slice(tci * TCH, (tci + 1) * TCH)
                        t0 = (tci * TCH) % S
                        tabsl = slice(t0, t0 + TCH)
                        ht = hst_pool.tile([P, DC * TCH], BF16, tag="hst")
                        hview = ht[:].rearrange("p (o t) -> p o t", o=DC)
                        if tci == 0:
                            # startup: weights on the SP DGE ring, hsT chunks
                            # on the Pool DGE ring -> the two streams transfer
                            # in parallel and the first q matmul starts ~2us
                            # in. dc-interleaved so quarter N+1 lands before
                            # the PE reaches its dc range.
                            for dq in range(4):
                                nc.sync.dma_start(
                                    wqg_sb[:, dq * 4:(dq + 1) * 4, :],
                                    wqg3[:, dq * 4:(dq + 1) * 4, :])
                                nc.gpsimd.dma_start(
                                    hview[:, dq * 4:(dq + 1) * 4, :],
                                    hsT3[:, dq * 4:(dq + 1) * 4, ts])
                                if dq == 0:
                                    nc.sync.dma_start(wkv_sb[:], wkv3)
                            nc.sync.dma_start(ident_sb[:], identd.ap())
                            nc.sync.dma_start(prot_sb[:], protd.ap())
                            nc.sync.dma_start(ck_sb[:], ck.ap())
                            nc.sync.dma_start(sk_sb[:], sk.ap())
                            nc.sync.dma_start(osel_sb[:], oseld.ap())
                            nc.sync.dma_start(v1_sb[:, :, HD:HD + 1],
                                              onesd.ap()[:, :, None])
                            # zero pads, after the ht-q DMAs so they don't
                            # delay the first chunk on the Pool DGE ring:
                            # scores contract K=128 with a zeroed half so
                            # each head pair partition-half stays independent
                            nc.gpsimd.memset(kv2lo[HD:P, :].bitcast(F32), 0.0)
                            nc.gpsimd.memset(kv2hi[0:HD, :].bitcast(F32), 0.0)
                            # v1 cols 65:128 zero -> psum rows 65:128 finite
                            nc.gpsimd.memset(v1_sb[:, :, HD + 1:P], 0.0)
                        else:
                            nc.gpsimd.dma_start(hview, hsT3[:, :, ts])
                        pss = [ps512.tile([P, TCH], F32, tag="ps512",
                                          name=f"psA{_m}") for _m in range(5)]
                        # kv accumulation trails q/gate by 2 dc-steps so the
                        # first matmuls only need wqg-q0 + ht-q0, not wkv.
                        # The previous chunk's rope/transpose drain at dc==5:
                        # its evictions are done by then, and the remaining
                        # ~12us of proj matmuls absorb the rot-psum slot waits
                        # (at the loop end they would stall the A->B junction)
                        for dc in range(DC + 2):
                            if dc == 5:
                                if pend_vt:
                                    pend_vt.pop(0)()
                                if pend_rope:
                                    pend_rope.pop(0)[0]()
                            if dc < DC:
                                for mt in range(1, 5):  # q pairs, gate pairs
                                    nc.tensor.matmul(
                                        pss[mt][:],
                                        lhsT=wqg_sb[:, dc, (mt - 1) * P:mt * P],
                                        rhs=ht[:, dc * TCH:(dc + 1) * TCH],
                                        start=(dc == 0),
                                        stop=(dc == DC - 1),
                                    )
                            if dc >= 2:
                                d2 = dc - 2
                                nc.tensor.matmul(
                                    pss[0][:],
                                    lhsT=wkv_sb[:, d2, :],
                                    rhs=ht[:, d2 * TCH:(d2 + 1) * TCH],
                                    start=(d2 == 0),
                                    stop=(d2 == DC - 1),
                                )
                        # evictions
                        nc.vector.tensor_copy(kv2lo[0:HD, ts], pss[0][0:HD, :])
                        vstg = vst.tile([HD, TCH], F32R, tag="vst")
                        nc.vector.tensor_copy(vstg[:], pss[0][HD:P, :])
                        nc.scalar.copy(qP_sb[:, 0, ts], pss[1][:])
                        nc.scalar.copy(qP_sb[:, 1, ts], pss[2][:])
                        for mo in range(2):
                            nc.scalar.activation(gsb[:, mo, ts], pss[3 + mo][:],
                                                 AF.Sigmoid)
                        # rope deferred one chunk: the rot matmuls land
                        # in the PE queue after the NEXT chunk's projection
                        # matmuls, whose evictions are then long done
                        def do_rope(ts=ts, tabsl=tabsl, pool=psrot, tag="rot"):
                            rope(kv2lo[:, ts], tabsl, HD, pool, tag)
                            nc.vector.tensor_copy(kv2hi[HD:P, ts],
                                                  kv2lo[0:HD, ts])
                            rope(qP_sb[:, 0, ts], tabsl, P, pool, tag)
                            rope(qP_sb[:, 1, ts], tabsl, P, pool, tag)
                        pend_rope.append((do_rope, ts, tabsl))
                        vstgs.append(vstg)
                        pend_vt.append(lambda tci=tci: do_vt(tci))
                    # prefetch the exp act table (different set than sigmoid)
                    # now, so it loads behind the chunk-7 sigmoids and stage
                    # B's first exp doesn't pay the ~1.3us load. ck_sb[0,0]
                    # is dead here: chunks 0/4 (the tabsl windows containing
                    # col 0) drained their ropes above, and the deferred
                    # chunk-7 rope reads cols 1536:2048 only.
                    nc.scalar.activation(ck_sb[0:1, 0:1], ck_sb[0:1, 1:2],
                                         AF.Exp)
                    while pend_vt:
                        pend_vt.pop(0)()
                    # drain all but the last chunk's rope; chunk 7 holds b1
                    # data that stage B only needs ~100us later, and draining
                    # its DVE ops here would sit in the DVE FIFO ahead of
                    # stage B's first exps. Stage B drains it on the pssc ring.
                    while len(pend_rope) > 1:
                        pend_rope.pop(0)[0]()
                    _, lts, ltab = pend_rope.pop(0)
                    leftover_rope = [
                        lambda pool, lts=lts, ltab=ltab: (
                            rope(kv2lo[:, lts], ltab, HD, pool, "pssc"),
                            nc.vector.tensor_copy(kv2hi[HD:P, lts],
                                                  kv2lo[0:HD, lts])),
                        lambda pool, lts=lts, ltab=ltab: rope(
                            qP_sb[:, 0, lts], ltab, P, pool, "pssc"),
                        lambda pool, lts=lts, ltab=ltab: rope(
                            qP_sb[:, 1, lts], ltab, P, pool, "pssc"),
                    ]
                # ---- stage B: attention + deferred normalize + stage C ----
                with (
                    nc.named_scope("stageB"),
                    tc.tile_pool(name="exp", bufs=4) as exp_pool,
                    tc.tile_pool(name="expi", bufs=3) as expi_pool,
                    tc.tile_pool(name="small", bufs=2) as small,
                    tc.tile_pool(name="wop", bufs=1) as wop,
                    tc.tile_pool(name="sgp", bufs=1) as sgp,
                    tc.tile_pool(name="evC", bufs=4) as evC,
                    tc.tile_pool(name="agp", bufs=1) as agp,
                    tc.tile_pool(name="pssc", bufs=4, space="PSUM") as pssc,
                    tc.tile_pool(name="psat", bufs=2, space="PSUM") as psat,
                ):
                    IW = 2 * TCH               # 1024-wide i-half
                    wo_sb = wop.tile([P, 2, D], BF16)
                    nc.sync.dma_start(wo_sb[:], wo3)
                    attnG_sb = agp.tile([P, 2, T], BF16)
                    den = sgp.tile([P, 2, IW], F32R)  # ping-pong by ihalf parity
                    # Pool memset: a DVE memset here would sit in the DVE
                    # FIFO ahead of stage B's first exps
                    nc.gpsimd.memset(den[0:HD, :, :].bitcast(F32), 0.0)

                    pending = []   # deferred normalize args

                    def emit_C(trange):
                        # C psum comes from the 512-wide pssc ring (no psat
                        # conflict with in-flight deferred normalizes)
                        for tt in trange:
                            tsl = slice(tt * P, (tt + 1) * P)
                            ev = evC.tile([P, D], BF16, tag="evC")
                            for oh in range(2):
                                for ii in range(2):
                                    o0 = oh * IW + ii * TCH
                                    ps = pssc.tile([P, TCH], F32, tag="pssc")
                                    for mc in range(2):
                                        nc.tensor.matmul(
                                            ps[:],
                                            lhsT=attnG_sb[:, mc, tsl],
                                            rhs=wo_sb[:, mc, o0:o0 + TCH],
                                            start=(mc == 0),
                                            stop=(mc == 1),
                                        )
                                    osl = slice(o0, o0 + TCH)
                                    if oh == 0:
                                        nc.vector.tensor_copy(ev[:, osl], ps[:])
                                    else:
                                        nc.scalar.copy(ev[:, osl], ps[:])
                            # alternate DGE rings so the final burst's out
                            # DMAs drain in parallel instead of 12us serial
                            eng = nc.sync if tt % 2 == 0 else nc.gpsimd
                            eng.dma_start(out.ap()[tsl, :], ev[:])

                    def emit_den(par, a_ps):
                        # denom is psum row 64 (ones-col accumulation); rows
                        # 65:128 are zeros. Copy rows 64:128 aligned into den
                        # (rows 0:64 pre-zeroed once) right at i-half end:
                        # DVE is idle at the boundary (its first exp chunk is
                        # jc=2), so this never blocks the next i-half's exps.
                        for ii in range(2):
                            isl = slice(ii * TCH, (ii + 1) * TCH)
                            nc.vector.tensor_copy(den[HD:P, par, isl],
                                                  a_ps[HD:P, isl])

                    def normalize(b, h, ih, par, a_ps):
                        # broadcast row 64 to all partitions via the row-64
                        # selector matmul, 1/x, then the gated muls (gate mul
                        # on the Pool engine). Runs deferred mid-next-ihalf
                        # so it doesn't collide with the boundary exps.
                        hp = (h % 2) * HD
                        mo = h // 2
                        osl = slice(b * S + ih * IW, b * S + (ih + 1) * IW)
                        rcp = small.tile([P, IW], F32, tag="rcp")
                        for ii in range(2):
                            isl = slice(ii * TCH, (ii + 1) * TCH)
                            bc_ps = pssc.tile([P, TCH], F32, tag="pssc")
                            nc.tensor.matmul(
                                bc_ps[:],
                                lhsT=osel_sb[:],
                                rhs=den[:, par, isl],
                                start=True, stop=True,
                            )
                            # reciprocal_approx_fast misbehaves at base
                            # partition 64; bc rows are all the denominator,
                            # so always use rows 0:64 (mixed-space mul allows
                            # base mismatch)
                            nc.vector.reciprocal_approx_fast(
                                out=rcp[0:HD, isl], in_=bc_ps[0:HD, :])
                        ag = attnG_sb[hp:hp + HD, mo, osl]
                        nc.vector.tensor_mul(
                            out=ag, in0=a_ps[0:HD, :], in1=rcp[0:HD, :])
                        nc.gpsimd.tensor_mul(
                            out=ag, in0=ag, in1=gsb[hp:hp + HD, mo, osl])

                    def flush():
                        while pending:
                            normalize(*pending.pop(0))

                    ihc = 0    # global i-half unit counter (den parity)
                    for b in range(B):
                        for ih in range(2):
                            for h in range(NH):
                                mo = h // 2
                                kvt = kv2lo if h % 2 == 0 else kv2hi
                                i0 = b * S + ih * IW
                                isl = slice(i0, i0 + IW)
                                a_ps = psat.tile([P, IW], F32, tag="psat")

                                def scores_exp(jc):
                                    jsl = slice(b * S + jc * P,
                                                b * S + (jc + 1) * P)
                                    dve = jc in DVE_JCS
                                    if dve:
                                        ex = expi_pool.tile([P, IW], I32,
                                                            tag="expi")
                                    else:
                                        ex = exp_pool.tile([P, IW], BF16,
                                                           tag="exp")
                                    for ii in range(2):
                                        isl = slice(ii * TCH, (ii + 1) * TCH)
                                        s_ps = pssc.tile([P, TCH], F32,
                                                         tag="pssc")
                                        nc.tensor.matmul(
                                            s_ps[:],
                                            lhsT=kvt[:, jsl],
                                            rhs=qP_sb[:, mo,
                                                      i0 + ii * TCH:
                                                      i0 + (ii + 1) * TCH],
                                            start=True,
                                            stop=True,
                                        )
                                        if dve:
                                            nc.vector.tensor_scalar(
                                                out=ex[:, isl], in0=s_ps[:],
                                                scalar1=AS_CONST,
                                                scalar2=BS_CONST,
                                                op0=ALU.mult, op1=ALU.add,
                                            )
                                        else:
                                            nc.scalar.activation(
                                                ex[:, isl], s_ps[:],
                                                AF.Exp, scale=0.125)
                                    return ex

                                def attn_acc(jc, ex):
                                    if ex.dtype == I32:
                                        # top 16 bits of the Schraudolph int32
                                        # ARE the bf16 exp value; emit per
                                        # psum-bank halves so the strided AP
                                        # survives codegen
                                        for ii in range(2):
                                            r = ex[:, ii * TCH:(ii + 1) * TCH]
                                            r = r.bitcast(BF16).rearrange(
                                                "p (n t) -> p n t", t=2)[:, :, 1]
                                            nc.tensor.matmul(
                                                a_ps[:, ii * TCH:(ii + 1) * TCH],
                                                lhsT=v1_sb[:, b * SJ + jc, :],
                                                rhs=r,
                                                start=(jc == 0),
                                                stop=(jc == SJ - 1),
                                            )
                                        return
                                    for ii in range(2):
                                        nc.tensor.matmul(
                                            a_ps[:, ii * TCH:(ii + 1) * TCH],
                                            lhsT=v1_sb[:, b * SJ + jc, :],
                                            rhs=ex[:, ii * TCH:(ii + 1) * TCH],
                                            start=(jc == 0),
                                            stop=(jc == SJ - 1),
                                        )

                                prev = scores_exp(0)
                                for jc in range(1, SJ):
                                    cur = scores_exp(jc)
                                    if jc == 6:
                                        flush()
                                    if jc == 10 and leftover_rope:
                                        # chunk-7 rope, deferred from stage A
                                        # (b1 data, needed ~8 i-halves later)
                                        leftover_rope.pop(0)(pssc)
                                    attn_acc(jc - 1, prev)
                                    prev = cur
                                attn_acc(SJ - 1, prev)
                                emit_den(ihc % 2, a_ps)
                                pending.append((b, h, ih, ihc % 2, a_ps))
                                ihc += 1
                                # interleave out-projection chunks: b0 tiles
                                # spread over all 8 b1 units, b1-ih0 tiles
                                # over the 4 b1-ih1 units (their heads all
                                # normalized once ih1's first flush ran)
                                if b == 1:
                                    s = ih * NH + h
                                    emit_C(range(2 * s, 2 * s + 2))
                                    if ih == 1 and h > 0:
                                        t0 = NT // 2 + (h - 1) * 3
                                        emit_C(range(t0, min(t0 + 3,
                                                             NT // 2 + 8)))
                    flush()
                    emit_C(range(NT - 8, NT))

    nc.compile()
    return nc


_NC_CACHE = None


def _get_nc(nreps=1):
    global _NC_CACHE
    if _NC_CACHE is None:
        _NC_CACHE = {}
    if nreps not in _NC_CACHE:
        _NC_CACHE[nreps] = build_nc(nreps)
    return _NC_CACHE[nreps]


def _dup_rows(tab64):
    """[64, S] -> [128, S] with both partition halves holding the table."""
    return np.ascontiguousarray(np.concatenate([tab64, tab64], axis=0))


def _prep_inputs(hidden_states, cos, sin, Wq, Wk, Wv, Wo):
    hs = np.asarray(hidden_states, dtype=np.float32)
    cos = np.asarray(cos, dtype=np.float32)
    sin = np.asarray(sin, dtype=np.float32)
    Wq = np.asarray(Wq, dtype=np.float32)
    Wk = np.asarray(Wk, dtype=np.float32)
    Wv = np.asarray(Wv, dtype=np.float32)
    Wo = np.asarray(Wo, dtype=np.float32)

    bf16 = mybir.dt.np(BF16)
    hsT = np.ascontiguousarray(hs.reshape(T, D).T).astype(bf16)

    cosT = cos.T                                     # [64, S]
    sinT = sin.T
    sin_signed = np.concatenate([-sinT[:HH], sinT[HH:]], axis=0)
    osel = np.zeros((P, P), np.float32)
    osel[HD, :] = 1.0
    prot = np.zeros((P, P), np.float32)
    for k in range(P):
        prot[k, k ^ HH] = 1.0
    common = {
        "hsT": hsT,
        "ck": _dup_rows(cosT).astype(bf16),
        "sk": _dup_rows(sin_signed).astype(bf16),
        "ident": np.eye(HD, dtype=np.float32),
        "osel": osel,
        "prot": prot,
        "ones": np.ones((P, B * SJ), mybir.dt.np(BF16)),
    }
    in_maps = []
    for c in range(NCORES):
        qcols = Wq[:, c * MQ:(c + 1) * MQ]
        gcols = Wq[:, H * HD + c * MQ: H * HD + (c + 1) * MQ]
        in_maps.append(
            {
                **common,
                "wqg": np.ascontiguousarray(
                    np.concatenate([qcols, gcols], axis=1)
                ).astype(bf16),
                "wkv": np.ascontiguousarray(
                    np.concatenate(
                        [Wk[:, c * HD:(c + 1) * HD], Wv[:, c * HD:(c + 1) * HD]],
                        axis=1,
                    )
                ).astype(bf16),
                "wo": np.ascontiguousarray(Wo[c * MQ:(c + 1) * MQ, :]).astype(bf16),
            }
        )
    return in_maps


def kernel(hidden_states, cos, sin, Wq, Wk, Wv, Wo, _trace=False, _trace_kwargs=None):
    nc = _get_nc()
    in_maps = _prep_inputs(hidden_states, cos, sin, Wq, Wk, Wv, Wo)
    res = run_bass_kernel_spmd(
        nc, in_maps, list(range(NCORES)), trace=_trace, **(_trace_kwargs or {})
    )
    total = res.results[0]["out"].astype(np.float32).copy()
    for c in range(1, NCORES):
        total += res.results[c]["out"]
    out = total.reshape(B, S, D)
    if _trace:
        kernel._last_results = res
    return out

